# revision 9
# baseline (speedup 1.0000x reference)
"""CRF-RNN mean-field iteration kernel for Trainium2 (8 NeuronCores).

Math (per batch b, NITERS=5):
    D_norm = W / W.sum(axis=1, keepdims)          # row-normalized affinity [n, n]
    qVals  = uniqs = seg.reshape(d, n)
    loop:  Q = softmax(qVals, axis=0)             # over class dim d=21
           seg_diff   = Q @ D_norm^T              # [d, n]
           seg_update = weights @ seg_diff
           qVals      = uniqs - seg_update

Sharding: batch b -> core pair (2b, 2b+1); each core owns half the output
positions (m rows of W). The contraction runs over all n, so W^T (contraction
index on partitions) is built on-device via PE transpose-matmuls against an
identity, quantized to fp8-e4m3, and kept resident in SBUF across all 5
iterations -- W is read from HBM exactly once. The main matmuls run in fp8
DoubleRow mode (256-wide contraction per pass). Row-normalization (1/rowsum,
accumulated for free during the fp32->fp8 cast on the Scalar engine) is
applied per-partition to the tiny seg_update output. Iteration 0 is emitted
interleaved with the (DMA-bound) transpose prepass so its matmuls hide under
the HBM reads.

Per iteration the pair exchanges its half of softmax(Q) via pairwise
AllGather, split into TWO chunks launched as soon as the first/second half of
the softmax tail completes, so the exchange latency hides under the next
iteration's own-half matmuls (a PE idle gap also drops the HAM clock to 1.2
GHz for ~3.5us, doubling matmul cost, so keeping the PE dense matters twice).
Exchange plumbing is spread across engine queues so no blocking wait sits in
front of latency-critical work: payload DMA on the vector ring (right after
the producing op), collective kicks on gpsimd, readbacks on sync, selects on
vector after the last tail. The instruction stream is identical on all cores
(SPMD): all own/partner asymmetry lives in host-side input permutations and a
tiny select-mask input.
"""

import os
import sys

for _p in ("/opt/trn_rl_repo",):
    if _p not in sys.path:
        sys.path.insert(0, _p)

import numpy as np

BS, D, RC = 4, 21, 64
N = RC * RC       # 4096 positions
NH = N // 2       # 2048 positions per core (own half)
NT = 32           # 128-wide position tiles (global)
NTO = 16          # own tiles
NT2 = 16          # 256-wide fp8 pair tiles (global)
SLABS = 16        # own-half m slabs of 128 rows
QPAD = 32         # class-dim padding for fp8 DoubleRow lhsT stride
NITERS = int(os.environ.get("CRF_NITERS", "5"))
ITER_FILLERS = int(os.environ.get("CRF_ITF", "4"))
NCORES = 8
RG = [[0, 1], [2, 3], [4, 5], [6, 7]]

LAST_EXEC_NS = None
_CACHE = {}


def _install_ntff_hook():
    """Best-effort registration of the axon NTFF profile hook (image antenv
    lacks axon_hooks, so trn_boot could not register it)."""
    try:
        import types

        if "antenv.axon_hooks" in sys.modules:
            return
        holder = [None]
        m = types.ModuleType("antenv.axon_hooks")
        m.set_axon_ntff_profile_hook = lambda h: holder.__setitem__(0, h)
        m.get_axon_ntff_profile_hook = lambda: holder[0]
        sys.modules["antenv.axon_hooks"] = m
        import antenv

        antenv.axon_hooks = m
        from trn_agent_boot.trn_boot import _ntff_profile_via_ctypes

        m.set_axon_ntff_profile_hook(
            _ntff_profile_via_ctypes("/opt/axon/libaxon_pjrt.so")
        )
    except Exception:
        pass


def _build(niters):
    from concourse import bacc, bass, tile, mybir

    fp32, fp16 = mybir.dt.float32, mybir.dt.float16
    sdt = mybir.dt.float8e4
    qpad = QPAD
    AF = mybir.ActivationFunctionType
    ALU = mybir.AluOpType
    ntile = NT2
    half = ntile // 2
    perf = mybir.MatmulPerfMode.DoubleRow
    CHK = half * qpad  # bytes per partition of one exchange chunk

    nc = bacc.Bacc(None, target_bir_lowering=False)

    w_in = nc.dram_tensor("w", (NH, N), fp32, kind="ExternalInput")
    segt_in = nc.dram_tensor("segt", (128, NT, D), fp32, kind="ExternalInput")
    wt_in = nc.dram_tensor("wt", (D, D), fp32, kind="ExternalInput")
    sel_in = nc.dram_tensor("sel", (128, 2), fp32, kind="ExternalInput")
    id_in = nc.dram_tensor("ident", (128, 128), fp32, kind="ExternalInput")
    out_t = nc.dram_tensor("out", (128, NTO, D), fp32, kind="ExternalOutput")

    n_ex = max(0, niters - 1)
    cc_ins = [
        nc.dram_tensor(f"cc_in{k}", (128, NTO * qpad), sdt, kind="Internal")
        for k in range(n_ex)
    ]
    cc_outs = [
        nc.dram_tensor(f"cc_out{k}", (2, 128, NTO * qpad), sdt, kind="Internal")
        for k in range(n_ex)
    ]

    with tile.TileContext(nc) as tc:
        with (
            tc.tile_pool(name="wt_res", bufs=1) as wt_res,
            tc.tile_pool(name="slab32", bufs=4) as slab32p,
            tc.tile_pool(name="slab8", bufs=2) as slab8p,
            tc.tile_pool(name="state", bufs=1) as state,
            tc.tile_pool(name="qt", bufs=2) as qtp,
            tc.tile_pool(name="work", bufs=2) as work,
            tc.tile_pool(name="ps_mm", bufs=1, space=bass.MemorySpace.PSUM) as ps_mm,
            tc.tile_pool(name="ps_misc", bufs=3, space=bass.MemorySpace.PSUM) as ps_misc,
            tc.tile_pool(name="ps_junk", bufs=1, space=bass.MemorySpace.PSUM) as ps_junk,
        ):
            # ---- small inputs.  segt goes first and alone on the scalar
            # ring (it gates the initial softmax); the other small inputs go
            # on the gpsimd ring so nothing queues ahead of segt or the W
            # slabs (sync ring).
            segt = state.tile([128, NT, D], fp32)
            nc.scalar.dma_start(segt[:], segt_in[:])
            id32 = state.tile([128, 128], fp32)
            nc.gpsimd.dma_start(id32[:], id_in[:])
            wt32 = state.tile([D, D], fp32)
            nc.gpsimd.dma_start(wt32[:], wt_in[:])
            selt = state.tile([128, 2], fp32)
            nc.gpsimd.dma_start(selt[:], sel_in[:])
            id_s = state.tile([128, 128], sdt)
            nc.vector.tensor_copy(id_s[:], id32[:])
            wt16 = state.tile([D, D], fp16)
            nc.gpsimd.tensor_copy(wt16[:], wt32[:])
            zbias = state.tile([128, 1], fp32)
            nc.gpsimd.memset(zbias[:], 0.0)
            # mask for predicated partner select: nonzero where slot1=partner
            selmask = state.tile([128, NTO * qpad], mybir.dt.uint8)
            nc.gpsimd.tensor_scalar_mul(
                selmask[:],
                selt[:, 1:2].broadcast_to((128, NTO * qpad)),
                1.0,
            )

            # ---- initial Q = softmax(uniqs) over all 32 tiles ------------
            ex0 = state.tile([128, NT, D], fp32)
            nc.scalar.activation(ex0[:], segt[:], AF.Exp, bias=zbias[:])
            ssum0 = state.tile([128, NT], fp32)
            nc.vector.reduce_sum(ssum0[:], ex0[:], axis=mybir.AxisListType.X)
            srecip0 = state.tile([128, NT], fp32)
            nc.vector.reciprocal(srecip0[:], ssum0[:])
            qt_own = qtp.tile([128, NTO, qpad], sdt, tag="qt_own", name="qt_own0")
            qt_par = qtp.tile([128, NTO, qpad], sdt, tag="qt_par", name="qt_par0")
            nc.vector.tensor_tensor(
                qt_own[:, :, 0:D],
                ex0[:, 0:NTO, :],
                srecip0[:, 0:NTO, None].broadcast_to((128, NTO, D)),
                ALU.mult,
            )
            nc.vector.tensor_tensor(
                qt_par[:, :, 0:D],
                ex0[:, NTO:NT, :],
                srecip0[:, NTO:NT, None].broadcast_to((128, NTO, D)),
                ALU.mult,
            )

            # ---- resident W^T (fp8, pair-interleaved for DoubleRow) ------
            # wt_mc[mc][p, t2, i, j] = W^T[256*t2 + 128*i + p, 512*mc + j]
            wt_mc = [
                wt_res.tile([128, NT2, 2, 512], sdt, tag=f"wtr{mc}", name=f"wt_mc{mc}")
                for mc in range(4)
            ]
            rs_colg = [
                state.tile([128, 4], fp32, tag=f"rscol{g}", name=f"rs_col{g}")
                for g in range(4)
            ]
            rs_recg = [
                state.tile([128, 4], fp32, tag=f"rsrec{g}", name=f"rs_rec{g}")
                for g in range(4)
            ]

            # scratch psum + filler matmuls: keep TensorE dense through DMA /
            # exchange waits so the HAM clock gate stays at full rate
            junk = ps_junk.tile([D, 512], fp32, name="junk")

            def fillers(n, rhs_fn):
                for f in range(n):
                    nc.tensor.matmul(
                        junk[:], id_s[:, 0:D], rhs_fn(f), start=True, stop=True
                    )

            def wt_filler_rhs(f):
                return wt_mc[f % 4][:, f % NT2, 0, :]

            def lhs_of(t, q_own, q_par):
                src = q_own if t < half else q_par
                j2 = t % half
                return src[:, 2 * j2 : 2 * j2 + 2, 0:D]

            def rhs_of(t, mc):
                return wt_mc[mc][:, t, :, :]

            class IterEmitter:
                """Emits one mean-field iteration in dependency-friendly
                pieces so matmuls, evacuations, the softmax tail, and the
                two exchange chunks pipeline across engines (and, for
                iteration 0, interleave with the prepass)."""

                def __init__(self, it, q_own, q_par, last):
                    self.it, self.q_own, self.q_par, self.last = it, q_own, q_par, last
                    self.pP = ps_mm.tile([D, NH], fp32, tag="pp", name=f"pp{it}")
                    self.ps16g = [None] * 4
                    self.pUTg = [None] * 4
                    self.qt_next = None
                    if not last:
                        self.qt_next = qtp.tile(
                            [128, NTO, qpad], sdt, tag="qt_own", name=f"qt_own{it+1}"
                        )

                def phase(self, mms):
                    for t, mc in mms:
                        nc.tensor.matmul(
                            self.pP[:, mc * 512 : (mc + 1) * 512],
                            lhs_of(t, self.q_own, self.q_par),
                            rhs_of(t, mc),
                            start=(t == 0),
                            stop=(t == ntile - 1),
                            perf_mode=perf,
                        )

                def evac(self, mc):
                    t16 = work.tile(
                        [D, 512], fp16, tag=f"ps16_{mc}", name=f"ps16_{self.it}_{mc}"
                    )
                    eng = nc.vector if mc % 2 == 0 else nc.scalar
                    if eng is nc.scalar:
                        nc.scalar.activation(
                            t16[:], self.pP[:, mc * 512 : (mc + 1) * 512], AF.Copy
                        )
                    else:
                        nc.vector.tensor_copy(
                            t16[:], self.pP[:, mc * 512 : (mc + 1) * 512]
                        )
                    self.ps16g[mc] = t16

                def ut(self, g):
                    pu = ps_misc.tile(
                        [128, 4 * D], fp32, tag="misc", name=f"pUT{self.it}_{g}"
                    )
                    for jj in range(4):
                        nc.tensor.matmul(
                            pu[:, jj * D : (jj + 1) * D],
                            self.ps16g[g][:, jj * 128 : (jj + 1) * 128],
                            wt16[:],
                            start=True,
                            stop=True,
                        )
                    self.pUTg[g] = pu

                def tail(self, g):
                    it, sl = self.it, slice(4 * g, 4 * g + 4)
                    upd = work.tile([128, 4, D], fp32, tag=f"upd{g}", name=f"upd{it}_{g}")
                    nc.vector.tensor_tensor(
                        upd[:],
                        self.pUTg[g][:].rearrange("p (a b) -> p a b", a=4),
                        rs_recg[g][:, :, None].broadcast_to((128, 4, D)),
                        ALU.mult,
                    )
                    qv = work.tile([128, 4, D], fp32, tag=f"qv{g}", name=f"qv{it}_{g}")
                    nc.vector.tensor_tensor(qv[:], segt[:, sl, :], upd[:], ALU.subtract)
                    if self.last:
                        nc.sync.dma_start(out_t[:, sl, :], qv[:])
                        return
                    exq = work.tile([128, 4, D], fp32, tag=f"exq{g}", name=f"exq{it}_{g}")
                    nc.scalar.activation(exq[:], qv[:], AF.Exp, bias=zbias[:])
                    ssum = work.tile([128, 4], fp32, tag=f"ssum{g}", name=f"ssum{it}_{g}")
                    nc.vector.reduce_sum(ssum[:], exq[:], axis=mybir.AxisListType.X)
                    srec = work.tile([128, 4], fp32, tag=f"srec{g}", name=f"srec{it}_{g}")
                    nc.vector.reciprocal(srec[:], ssum[:])
                    nc.vector.tensor_tensor(
                        self.qt_next[:, sl, 0:D],
                        exq[:],
                        srec[:, :, None].broadcast_to((128, 4, D)),
                        ALU.mult,
                    )

                # -- the full-qt exchange: one collective per iteration (a
                # collective has ~5us fixed dispatch cost and two of them
                # serialize on the CC core, so chunking loses).  The two
                # readback kicks go on different queues (sync + scalar) so
                # they issue in parallel the moment the cc completes.
                def exchange(self):
                    it = self.it
                    nc.sync.dma_start(cc_ins[it][:], self.qt_next[:])
                    nc.gpsimd.collective_compute(
                        "AllGather",
                        ALU.bypass,
                        replica_groups=RG,
                        ins=[cc_ins[it][:].opt()],
                        outs=[cc_outs[it][:].opt()],
                    )
                    qt_par_next = qtp.tile(
                        [128, NTO, qpad], sdt, tag="qt_par", name=f"qt_par{it+1}"
                    )
                    g1 = work.tile([128, NTO * qpad], sdt, tag="g1", name=f"g1_{it}")
                    nc.sync.dma_start(
                        qt_par_next[:].rearrange("p a b -> p (a b)"), cc_outs[it][0][:]
                    )
                    nc.scalar.dma_start(g1[:], cc_outs[it][1][:])
                    nc.vector.copy_predicated(
                        qt_par_next[:].rearrange("p a b -> p (a b)"),
                        selmask[:],
                        g1[:],
                    )
                    return qt_par_next

            # ---- prepass (slabs, transpose, rowsum) + iteration 0 --------
            em = IterEmitter(0, qt_own, qt_par, last=(niters == 1))
            for ms in range(SLABS):
                w32 = slab32p.tile([128, N], fp32, tag="w32", name=f"w32_{ms}")
                nc.sync.dma_start(w32[:], w_in[ms * 128 : (ms + 1) * 128, :])
                w8 = slab8p.tile([128, N], sdt, tag="w8", name=f"w8_{ms}")
                nc.scalar.activation(
                    w8[:], w32[:], AF.Copy,
                    accum_out=rs_colg[ms // 4][:, ms % 4 : ms % 4 + 1],
                )
                mc, col = ms // 4, (ms % 4) * 128
                for g in range(8):
                    ptp = ps_misc.tile([128, 512], fp32, tag="misc", name=f"ptp{ms}_{g}")
                    for k2 in range(4):
                        nt = 4 * g + k2
                        nc.tensor.matmul(
                            ptp[:, k2 * 128 : (k2 + 1) * 128],
                            w8[:, nt * 128 : (nt + 1) * 128],
                            id_s[:],
                            start=True,
                            stop=True,
                        )
                    dst = wt_mc[mc][:, 2 * g : 2 * g + 2, :, col : col + 128]
                    src = ptp[:].rearrange("p (a b c) -> p a b c", a=2, b=2)
                    # spread psum evacuations: scalar takes every 4th
                    if g % 4 == 3:
                        nc.scalar.activation(dst, src, AF.Copy)
                    else:
                        nc.vector.tensor_copy(dst, src)
                fillers(16, lambda f: w8[:, (f % 8) * 512 : (f % 8) * 512 + 512])
                if ms % 4 == 3:
                    g = ms // 4
                    nc.vector.reciprocal(rs_recg[g][:], rs_colg[g][:])
                    em.phase([(t, g) for t in range(ntile)])
                    em.evac(g)
                    if g >= 1:
                        em.ut(g - 1)
                        em.tail(g - 1)
            em.ut(3)
            em.tail(3)
            if niters > 1:
                qt_par = em.exchange()
                qt_own = em.qt_next

            # ---- iterations 1..niters-1 ---------------------------------
            # Tensor-queue emission order matters: the queue is in-order, so
            # ut(g) (which waits on evac(g), a vector/scalar op) is emitted
            # two partner phases after phase g's stop -- by then the evac has
            # long completed and the queue never blocks mid-stream.
            for it in range(1, niters):
                em = IterEmitter(it, qt_own, qt_par, last=(it == niters - 1))
                # own tiles: no exchange dependency; bridges the exchange
                em.phase([(t, mc) for t in range(half) for mc in range(4)])
                if ITER_FILLERS:
                    fillers(ITER_FILLERS, wt_filler_rhs)
                em.phase([(t, 0) for t in range(half, ntile)])
                em.evac(0)
                em.phase([(t, 1) for t in range(half, ntile)])
                em.evac(1)
                em.phase([(t, 2) for t in range(half, ntile)])
                em.evac(2)
                em.ut(0)
                em.tail(0)
                em.phase([(t, 3) for t in range(half, ntile)])
                em.evac(3)
                em.ut(1)
                em.tail(1)
                em.ut(2)
                em.tail(2)
                em.ut(3)
                em.tail(3)
                if it < niters - 1:
                    qt_par = em.exchange()
                    qt_own = em.qt_next

    nc.compile()
    return nc


def _get_nc(niters):
    if niters not in _CACHE:
        _CACHE[niters] = _build(niters)
    return _CACHE[niters]


def kernel(seg, W, weights):
    global LAST_EXEC_NS
    assert seg.shape == (BS, D, RC, RC) and W.shape == (BS, N, N)
    trace = bool(os.environ.get("BASS_TRACE"))
    if trace:
        _install_ntff_hook()

    from concourse.bass_utils import run_bass_kernel_spmd

    nc = _get_nc(NITERS)

    seg32 = np.ascontiguousarray(seg, dtype=np.float32)
    W32 = np.ascontiguousarray(W, dtype=np.float32)
    wt_np = np.ascontiguousarray(weights.T, dtype=np.float32)
    id_np = np.eye(128, dtype=np.float32)

    in_maps = []
    for c in range(NCORES):
        b, h = c // 2, c % 2
        own = slice(NH * h, NH * h + NH)
        par = slice(NH * (1 - h), NH * (1 - h) + NH)
        Wb = W32[b]
        w_np = np.ascontiguousarray(
            np.concatenate([Wb[own, own], Wb[own, par]], axis=1)
        )
        st = seg32[b].reshape(D, N).T  # [n, d]
        st_perm = np.concatenate([st[own], st[par]], axis=0)
        segt_np = np.ascontiguousarray(
            st_perm.reshape(NT, 128, D).transpose(1, 0, 2)
        )
        sel_np = np.zeros((128, 2), np.float32)
        sel_np[:, 0] = float(h)       # gather slot (1-h) = partner
        sel_np[:, 1] = float(1 - h)
        in_maps.append(
            {"w": w_np, "segt": segt_np, "wt": wt_np, "sel": sel_np, "ident": id_np}
        )

    res = run_bass_kernel_spmd(
        nc, in_maps, core_ids=list(range(NCORES)), trace=trace
    )
    LAST_EXEC_NS = res.exec_time_ns

    out = np.empty((BS, D, N), np.float32)
    for c in range(NCORES):
        b, h = c // 2, c % 2
        qv = res.results[c]["out"]  # [128, NTO, D]
        block = qv.transpose(2, 1, 0).reshape(D, NH)
        out[b][:, NH * h : NH * h + NH] = block
    return out.reshape(BS, D, RC, RC)


if __name__ == "__main__":
    rng = np.random.default_rng(0)
    seg = rng.standard_normal((BS, D, RC, RC)).astype(np.float32)
    W = rng.random((BS, N, N), dtype=np.float32)
    weights = rng.standard_normal((D, D)).astype(np.float32)
    out = kernel(seg=seg, W=W, weights=weights)
    print("out", out.shape, out.dtype, float(np.abs(out).mean()))
